# revision 1
# baseline (speedup 1.0000x reference)
"""Trainium2 Bass kernel for ChunkedTGnnModel (2-layer GCN over temporal chunks).

Math: the reference flattens each temporal chunk to a [128000, 64] slab
(row u = node*128 + t_local) while edges are replicated per-timestep with
t-major offsets (tl*N + v). Both live in the same flat index space, so the
per-chunk operator is block-diagonal: 128 consecutive 1000-row blocks of the
slab each get the same dense normalized adjacency A_hat [1000 x 1000]:

    out = relu(blockdiag(A_hat) @ (slab @ W1) + b1)   (then layer 2 same)

Sharding: 8 cores = 4 chunks x 2 node-halves; each core owns a contiguous
[64000, 64] slab piece (64 blocks = 32 block-pairs).

Per block-pair (b, b+1), per layer, on-chip (all matmul operands fp16,
accumulation fp32 in PSUM):
  A-type:  G.T = lhsT.T @ AT  with lhsT = slab tiles [128 rho, 2x64 (blk,d)]
           -> feature-major G.T [128 (blk,d), 1000 dest] in PSUM
  W-fold:  lhsT = G.T chunk [128 feats, <=128 dest rows], rhs = blockdiag(W)
           -> row-major H [dest rows, 2x64 (blk,dout)] in PSUM
  epilogue: DVE bias add + ACT relu -> fp16 tiles (= next layer's lhsT)
"""
import sys
import numpy as np

sys.path.insert(0, '/opt/trn_rl_repo')

import concourse.bass as bass  # noqa: E402
import concourse.bacc as bacc  # noqa: E402
import concourse.mybir as mybir  # noqa: E402
import concourse.tile as tile  # noqa: E402
from concourse.bass_utils import run_bass_kernel_spmd  # noqa: E402

N, T, D = 1000, 512, 64
CS = 128                 # timesteps per chunk
NCORES = 8
ROWS = 64000             # slab rows per core (64 blocks x 1000)
PAIRS = 32
RHO = [(128 * j, min(128, N - 128 * j)) for j in range(8)]   # (start, rows)
DEST = [(0, 512), (512, 488)]                                # A-type dest chunks

_prog = None
LAST_RESULTS = None


def _build_program(skip=frozenset()):
    nc = bacc.Bacc(None)
    xin = nc.declare_dram_parameter("xin", [ROWS, D], mybir.dt.float16, isOutput=False)
    at = nc.declare_dram_parameter("at", [N, N], mybir.dt.float16, isOutput=False)
    wt1 = nc.declare_dram_parameter("wt1", [128, 128], mybir.dt.float16, isOutput=False)
    wt2 = nc.declare_dram_parameter("wt2", [128, 128], mybir.dt.float16, isOutput=False)
    bs1 = nc.declare_dram_parameter("bs1", [128, 512], mybir.dt.float32, isOutput=False)
    bs2 = nc.declare_dram_parameter("bs2", [128, 512], mybir.dt.float32, isOutput=False)
    xout = nc.declare_dram_parameter("xout", [ROWS, D], mybir.dt.float32, isOutput=True)

    with tile.TileContext(nc) as tc:
        with tc.tile_pool(name="const", bufs=1) as cpool, \
             tc.tile_pool(name="work", bufs=2) as wpool, \
             tc.tile_pool(name="gps_pool", bufs=2, space="PSUM") as gpool, \
             tc.tile_pool(name="hps_pool", bufs=2, space="PSUM") as hpool:

            # [64 blocks, 1000 rows, 64 feats] views of the slab in HBM
            xin_b = xin.rearrange("(blk r) d -> blk r d", r=N)
            xout_b = xout.rearrange("(blk r) d -> blk r d", r=N)

            # per-pair live state: xt/h1/o2 tiles, g psum tiles
            st = {}

            def load_xt(p):
                # xt_all column layout: col = j*128 + b*64 + d, so lhsT for
                # rho-tile j is the contiguous slice [128j, 128j+128).
                b0 = 2 * p
                xt_all = wpool.tile([128, 1024], mybir.dt.float16,
                                    name="xt_all", tag="xt_all")
                if "indma" not in skip:
                    for bi in range(2):
                        dst = xt_all.rearrange("p (j w) -> p j w", w=128)[
                            :, 0:7, 64 * bi:64 * bi + 64]
                        src = xin_b[b0 + bi, 0:896, :].rearrange(
                            "(j i) d -> i j d", j=7)
                        nc.sync.dma_start(dst, src)
                        nc.sync.dma_start(
                            xt_all[0:104, 896 + 64 * bi:896 + 64 * bi + 64],
                            xin_b[b0 + bi, 896:1000, :])
                st[p] = {"xt": xt_all}

            def stage_A(p, li):
                """A-type matmuls for layer li of pair p -> g psum tiles."""
                if li == 0:
                    xt_all = st[p]["xt"]
                    def lhsT_of(j):
                        return xt_all[0:RHO[j][1], 128 * j:128 * j + 128]
                else:
                    h1 = st[p]["h1"]
                    def lhsT_of(j):
                        return h1[j // 4][0:RHO[j][1],
                                          128 * (j % 4):128 * (j % 4) + 128]
                gps = []
                for ci, (c0, cw) in enumerate(DEST):
                    gp = gpool.tile([128, cw], mybir.dt.float32,
                                    name=f"gps{ci}", tag=f"gps{ci}")
                    if "atype" not in skip:
                        for j, (r0, rj) in enumerate(RHO):
                            nc.tensor.matmul(gp[:, :], lhsT_of(j),
                                             at_t[j][:, c0:c0 + cw],
                                             start=(j == 0), stop=(j == 7))
                    gps.append(gp)
                st[p][f"g{li}"] = gps

            def stage_W(p, li):
                """psum->sbuf copies, W-fold matmuls, bias+relu for layer li."""
                gps = st[p].pop(f"g{li}")
                out_dtype = mybir.dt.float16 if li == 0 else mybir.dt.float32
                gsb = wpool.tile([128, N], mybir.dt.float16,
                                 name=f"gsb{li}", tag=f"gsb{li}")
                if "copies" not in skip:
                    # split across ACT and DVE to balance engine load
                    nc.scalar.copy(gsb[:, 0:512], gps[0][:, :])
                    nc.vector.tensor_copy(gsb[:, 512:1000], gps[1][:, :])

                hps = [hpool.tile([128, 512], mybir.dt.float32,
                                  name=f"hps{t}", tag=f"hps{t}") for t in range(2)]
                if "wfold" not in skip:
                    for ci, (c0c, rci) in enumerate(RHO):
                        t_, o = ci // 4, 128 * (ci % 4)
                        nc.tensor.matmul(hps[t_][0:rci, o:o + 128],
                                         gsb[:, c0c:c0c + rci], wt_t[li][:, :],
                                         start=True, stop=True)
                outs = [wpool.tile([128, 512], out_dtype,
                                   name=f"ho{li}_{t}", tag=f"ho{li}_{t}")
                        for t in range(2)]
                # coarse epilogue regions: (bank, rows, col0, col1); the last
                # delta-chunk only has 104 valid rows so it gets its own op.
                regions = [(0, 128, 0, 512), (1, 128, 0, 384), (1, 104, 384, 512)]
                for t_, rr, c0r, c1r in regions:
                    if "bias" not in skip:
                        nc.vector.tensor_add(hps[t_][0:rr, c0r:c1r],
                                             hps[t_][0:rr, c0r:c1r],
                                             bs_t[li][0:rr, 0:c1r - c0r])
                    if "relu" not in skip:
                        nc.scalar.activation(outs[t_][0:rr, c0r:c1r],
                                             hps[t_][0:rr, c0r:c1r],
                                             mybir.ActivationFunctionType.Relu)
                st[p]["h1" if li == 0 else "o2"] = outs

            def store_out(p):
                o2 = st[p].pop("o2")
                b0 = 2 * p
                if "outdma" not in skip:
                    o2v = [t_.rearrange("p (c b d) -> p c b d", c=4, b=2)
                           for t_ in o2]
                    for bi in range(2):
                        # split 3 on SWDGE (Pool) / 3 on HWDGE (SP)
                        eng = nc.gpsimd if bi == 0 else nc.sync
                        dstA = xout_b[b0 + bi, 0:512, :].rearrange(
                            "(c i) d -> i c d", c=4)
                        eng.dma_start(dstA, o2v[0][:, :, bi, :])
                        dstB = xout_b[b0 + bi, 512:896, :].rearrange(
                            "(c i) d -> i c d", c=3)
                        eng.dma_start(dstB, o2v[1][:, 0:3, bi, :])
                        eng.dma_start(
                            xout_b[b0 + bi, 896:1000, :],
                            o2v[1][0:104, 3, bi, :])
                del st[p]

            # prologue: first pair's input before the constants so the first
            # A-type matmuls start as early as possible
            # constants go over the Pool/SWDGE path so they don't serialize
            # against the pair input loads on HWDGE
            load_xt(0)
            at_t = []
            for j, (r0, rj) in enumerate(RHO):
                t_ = cpool.tile([rj, N], mybir.dt.float16, name=f"at{j}")
                eng = nc.gpsimd if j % 2 == 0 else nc.sync
                eng.dma_start(t_[:, :], at[r0:r0 + rj, :])
                at_t.append(t_)
            wt_t = []
            for li, wsrc in enumerate((wt1, wt2)):
                w_ = cpool.tile([128, 128], mybir.dt.float16, name=f"wt{li}")
                nc.gpsimd.dma_start(w_[:, :], wsrc[:, :])
                wt_t.append(w_)
            bs_t = []
            for li, bsrc in enumerate((bs1, bs2)):
                b_ = cpool.tile([128, 512], mybir.dt.float32, name=f"bst{li}")
                nc.gpsimd.dma_start(b_[:, :], bsrc[:, :])
                bs_t.append(b_)

            stage_A(0, 0)
            # software-pipelined steady state: every PE stall window is
            # covered by >=3us of independent A-type work from another pair
            for p in range(PAIRS + 1):
                if p + 1 < PAIRS:
                    load_xt(p + 1)
                if p < PAIRS:
                    stage_W(p, 0)
                if p + 1 < PAIRS:
                    stage_A(p + 1, 0)
                if p >= 1:
                    stage_W(p - 1, 1)
                    store_out(p - 1)
                if p < PAIRS:
                    stage_A(p, 1)

    nc.compile()
    return nc


def _host_prep(x, edge_index, W1, b1, W2, b2):
    x = np.ascontiguousarray(np.asarray(x, dtype=np.float32))
    ei = np.asarray(edge_index)
    row, col = ei[0], ei[1]
    deg = np.zeros(N, np.float32)
    np.add.at(deg, col, 1.0)
    deg += 1.0
    dinv = (1.0 / np.sqrt(deg)).astype(np.float32)
    A = np.zeros((N, N), np.float32)
    np.add.at(A, (col, row), (dinv[row] * dinv[col]).astype(np.float32))
    A[np.arange(N), np.arange(N)] += dinv * dinv
    AT16 = np.ascontiguousarray(A.T).astype(np.float16)

    wts = []
    for W in (W1, W2):
        wt = np.zeros((128, 128), np.float16)
        wt[:64, :64] = np.asarray(W).astype(np.float16)
        wt[64:, 64:] = np.asarray(W).astype(np.float16)
        wts.append(wt)
    bss = [np.ascontiguousarray(
        np.broadcast_to(np.tile(np.asarray(b, np.float32), 8), (128, 512)))
        for b in (b1, b2)]

    x16 = x.astype(np.float16)
    slabs = []
    for k in range(NCORES):
        c, hf = k // 2, k % 2
        slab = np.ascontiguousarray(
            x16[500 * hf:500 * hf + 500, 128 * c:128 * (c + 1), :]).reshape(ROWS, D)
        slabs.append(slab)
    return AT16, wts, bss, slabs


def kernel(x, edge_index, W1, b1, W2, b2):
    global _prog, LAST_RESULTS
    if _prog is None:
        _prog = _build_program()
    nc = _prog

    AT16, wts, bss, slabs = _host_prep(x, edge_index, W1, b1, W2, b2)
    in_maps = [{"xin": slabs[k], "at": AT16,
                "wt1": wts[0], "wt2": wts[1],
                "bs1": bss[0], "bs2": bss[1]} for k in range(NCORES)]

    LAST_RESULTS = run_bass_kernel_spmd(nc, in_maps, core_ids=list(range(NCORES)))

    out = np.empty((N, T, D), np.float32)
    for k in range(NCORES):
        c, hf = k // 2, k % 2
        r = LAST_RESULTS.results[k]["xout"].reshape(500, CS, D)
        out[500 * hf:500 * hf + 500, 128 * c:128 * (c + 1), :] = r
    return out



# revision 21
# speedup vs baseline: 1.9654x; 1.9654x over previous
"""Trainium2 Bass kernel for ChunkedTGnnModel (2-layer GCN over temporal chunks).

Math: the reference flattens each temporal chunk to a [128000, 64] slab
(row u = node*128 + t_local) while edges are replicated per-timestep with
t-major offsets (tl*N + v). Both live in the same flat index space, so the
per-chunk operator is block-diagonal: 128 consecutive 1000-row blocks of the
slab each get the same dense normalized adjacency A_hat [1000 x 1000]:

    out = relu(blockdiag(A_hat) @ (slab @ W1) + b1)   (then layer 2 same)

Key optimization vs the fp16 version: A_hat = D^-1/2 (Adj + I) D^-1/2 where
Adj+I has small-integer entries that are EXACT in fp8e4 (e4m3). We fold the
left/right D^-1/2 into the operands (host pre-scales X by dinv; dest-side
dinv^2 is folded into the PSUM->SBUF copy before the W-fold), so the big
A-type matmuls run in fp8 DoubleRow mode (256-deep contraction, 0.5
cycles/row = 4x fp16 PE throughput) with NO quantization error from the
adjacency itself. Layer-1 input additionally ships as an fp8 (hi, lo)
mantissa-split pair (costs nothing: host prep), halving its quantization
error; layer-2 re-quantizes on-chip in a single pass (measured end-to-end
rel err ~1.1e-2 vs the 2e-2 gate).

Bias is injected exactly by pre-loading the W-fold PSUM banks with the bias
tile and accumulating the matmuls on top (start=False).

Sharding: 8 cores = 4 chunks x 2 node-halves; each core owns a contiguous
[64000, 64] slab piece (64 blocks = 32 block-pairs). Output is written fp16
and upconverted on host.
"""
import sys
import numpy as np

sys.path.insert(0, '/opt/trn_rl_repo')

import concourse.bass as bass  # noqa: E402
import concourse.bacc as bacc  # noqa: E402
import concourse.mybir as mybir  # noqa: E402
import concourse.tile as tile  # noqa: E402
from concourse.bass_utils import run_bass_kernel_spmd  # noqa: E402

try:
    import ml_dtypes
    F8 = ml_dtypes.float8_e4m3
except ImportError:  # pragma: no cover
    F8 = None

N, T, D = 1000, 512, 64
CS = 128                 # timesteps per chunk
NCORES = 8
ROWS = 64000             # slab rows per core (64 blocks x 1000)
PAIRS = 32
NPAD = 1024              # padded src-node count (8 x 128)
# A-type moving chunks: (psum tile, col offset in tile, global c0, width)
A_CHUNKS = [(0, 0, 0, 256), (0, 256, 256, 256), (1, 0, 512, 256), (1, 256, 768, 232)]

COMP1 = True             # layer-1 fp8 hi+lo compensation (host-side split)
COMP2 = False            # layer-2 on-chip hi+lo compensation

_prog = None
LAST_RESULTS = None

F8D = mybir.dt.float8e4
DR = mybir.MatmulPerfMode.DoubleRow


def _build_program(skip=frozenset()):
    nc = bacc.Bacc(None)
    XW = 2 * NPAD if COMP1 else NPAD
    xin = nc.declare_dram_parameter("xin", [PAIRS * 128, XW], F8D, isOutput=False)
    at8 = nc.declare_dram_parameter("at8", [128, 8 * N], F8D, isOutput=False)
    wt1 = nc.declare_dram_parameter("wt1", [128, 128], mybir.dt.float16, isOutput=False)
    wt2 = nc.declare_dram_parameter("wt2", [128, 128], mybir.dt.float16, isOutput=False)
    db1 = nc.declare_dram_parameter("db1", [128, 1024], mybir.dt.float32, isOutput=False)
    db2 = nc.declare_dram_parameter("db2", [128, 1], mybir.dt.float32, isOutput=False)
    dsq = nc.declare_dram_parameter("dsq", [128, N], mybir.dt.float16, isOutput=False)
    dv1 = nc.declare_dram_parameter("dv1", [128, N], mybir.dt.float16, isOutput=False)
    # raw per-pair layer-2 output, transposed [pair*128 (blk,d), dest];
    # host un-permutes
    xout = nc.declare_dram_parameter("xout", [PAIRS * 128, N], mybir.dt.float16,
                                     isOutput=True)

    with tile.TileContext(nc) as tc:
        with tc.tile_pool(name="const", bufs=1) as cpool, \
             tc.tile_pool(name="work", bufs=2) as wpool, \
             tc.tile_pool(name="gps_pool", bufs=2, space="PSUM") as gpool, \
             tc.tile_pool(name="hps_pool", bufs=1, space="PSUM") as hpool:

            xhi_v = xhi.rearrange("(p q) f -> p q f", q=128)
            if COMP1:
                xlo_v = xlo.rearrange("(p q) f -> p q f", q=128)
            xout_v = xout.rearrange("(p q) v -> p q v", q=128)

            st = {}

            def load_xt(p):
                th = wpool.tile([128, NPAD], F8D, name="xth", tag="xth")
                if "indma" not in skip:
                    nc.sync.dma_start(th[:, :], xhi_v[p])
                tl_ = None
                if COMP1:
                    tl_ = wpool.tile([128, NPAD], F8D, name="xtl", tag="xtl")
                    if "indma" not in skip:
                        nc.sync.dma_start(tl_[:, :], xlo_v[p])
                st[p] = {"xt": (th, tl_)}

            def stage_A(p, li):
                """fp8 DoubleRow A-type matmuls for layer li -> g psum tiles."""
                if li == 0:
                    srcs = [t_ for t_ in st[p]["xt"] if t_ is not None]
                else:
                    srcs = [t_ for t_ in st[p]["h"] if t_ is not None]
                gps = [gpool.tile([128, 512], mybir.dt.float32,
                                  name="gps0", tag="gps0"),
                       gpool.tile([128, 488], mybir.dt.float32,
                                  name="gps1", tag="gps1")]
                nsteps = 4 * len(srcs)
                k = 0
                for src in srcs:
                    sv = src.rearrange("q (jj i f) -> q jj i f", jj=4, i=2)
                    for jj in range(4):
                        for (t_, col, c0, cw) in A_CHUNKS:
                            if "atype" in skip:
                                continue
                            # one accumulation group per PSUM bank: start
                            # zeroes the whole 2KB zero-region, so only the
                            # first matmul of each bank may set it
                            nc.tensor.matmul(
                                gps[t_][:, col:col + cw],
                                sv[:, jj],
                                at8_t[:, 2 * jj:2 * jj + 2, c0:c0 + cw],
                                start=(k == 0 and col == 0),
                                stop=(k == nsteps - 1 and col != 0),
                                perf_mode=DR)
                        k += 1
                st[p][f"g{li}"] = gps

            def stage_W(p, li):
                """scaled psum->sbuf copy, bias, W-folds, relu."""
                gps = st[p].pop(f"g{li}")
                drow = dsq_t if li == 0 else dv1_t
                gsb = wpool.tile([128, N], mybir.dt.float16,
                                 name=f"gsb{li}", tag=f"gsb{li}")
                # fold the dest-side dinv^2 (layer 1) / dinv (layer 2) into
                # the mandatory PSUM->SBUF copy
                if "copies" not in skip:
                    nc.vector.tensor_tensor(gsb[:, 0:512], gps[0][:, :],
                                            drow[:, 0:512], mybir.AluOpType.mult)
                    nc.vector.tensor_tensor(gsb[:, 512:N], gps[1][:, :],
                                            drow[:, 512:N], mybir.AluOpType.mult)

                if li == 0:
                    # row-major W-fold [dest, (blk,f)]; exact bias by psum
                    # preload (dinv[dest]*b1[f]) accumulated by the matmuls
                    hps = [hpool.tile([128, 512], mybir.dt.float32,
                                      name=f"h1_{t}", tag=f"h1_{t}")
                           for t in range(2)]
                    if "preload" not in skip:
                        nc.vector.tensor_copy(hps[0][:, :], db1_t[:, 0:512])
                        nc.scalar.copy(hps[1][:, :], db1_t[:, 512:1024])
                    for c in range(8):
                        rci = 128 if c < 7 else 104
                        t_, o = c // 4, 128 * (c % 4)
                        if "wfold" in skip:
                            continue
                        nc.tensor.matmul(hps[t_][0:rci, o:o + 128],
                                         gsb[:, 128 * c:128 * c + rci],
                                         wt_t[0][:, :],
                                         start=False, stop=True,
                                         skip_group_check=True)
                    regions = [(0, 128, 0, 512), (1, 128, 0, 384), (1, 104, 384, 512)]
                    hq = wpool.tile([128, NPAD], F8D, name="hq", tag="hq")
                    # zero the (src >= 1000) pad rows read by layer-2 lhsT
                    nc.gpsimd.memset(hq[96:128, 896:NPAD], 0)
                    for t_, rr, c0r, c1r in regions:
                        if "relu" in skip:
                            continue
                        nc.scalar.activation(hq[0:rr, 512 * t_ + c0r:512 * t_ + c1r],
                                             hps[t_][0:rr, c0r:c1r],
                                             mybir.ActivationFunctionType.Relu)
                    lq = None
                    if COMP2:
                        lq = wpool.tile([128, NPAD], F8D, name="lq", tag="lq")
                        nc.gpsimd.memset(lq[96:128, 896:NPAD], 0)
                        for t_, rr, c0r, c1r in regions:
                            nc.gpsimd.scalar_tensor_tensor(
                                lq[0:rr, 512 * t_ + c0r:512 * t_ + c1r],
                                hps[t_][0:rr, c0r:c1r], 0.0,
                                hq[0:rr, 512 * t_ + c0r:512 * t_ + c1r],
                                op0=mybir.AluOpType.max,
                                op1=mybir.AluOpType.subtract)
                    st[p]["h"] = (hq, lq)
                else:
                    # transposed W-fold: out H.T [(blk,f), dest]; bias b2[f]
                    # is then per-partition and fuses into the ACT relu
                    hps = [hpool.tile([128, 512], mybir.dt.float32,
                                      name="h2_0", tag="h2_0"),
                           hpool.tile([128, 488], mybir.dt.float32,
                                      name="h2_1", tag="h2_1")]
                    if "wfold" not in skip:
                        nc.tensor.matmul(hps[0][:, :], wt_t[1][:, :],
                                         gsb[:, 0:512], start=True, stop=True)
                        nc.tensor.matmul(hps[1][:, :], wt_t[1][:, :],
                                         gsb[:, 512:N], start=True, stop=True)
                    outs = [wpool.tile([128, 512], mybir.dt.float16,
                                       name="o2_0", tag="o2_0"),
                            wpool.tile([128, 488], mybir.dt.float16,
                                       name="o2_1", tag="o2_1")]
                    for t in range(2):
                        if "relu" in skip:
                            continue
                        nc.scalar.activation(outs[t][:, :], hps[t][:, :],
                                             mybir.ActivationFunctionType.Relu,
                                             bias=db2_t[:, 0:1])
                    st[p]["o2"] = outs

            def store_out(p):
                o2 = st[p].pop("o2")
                if "outdma" not in skip:
                    # raw contiguous tile dumps; host un-permutes
                    nc.sync.dma_start(xout_v[p, :, 0:512], o2[0][:, :])
                    nc.sync.dma_start(xout_v[p, :, 512:N], o2[1][:, :])
                del st[p]

            # prologue: first pair's input first so A-type starts early;
            # constants go over the Pool/SWDGE path to avoid serializing
            # against pair loads on HWDGE
            load_xt(0)
            at8_c = cpool.tile([128, 8 * N], F8D, name="at8")
            nc.sync.dma_start(at8_c[:, 0:4 * N], at8[:, 0:4 * N])
            nc.sync.dma_start(at8_c[:, 4 * N:8 * N], at8[:, 4 * N:8 * N])
            at8_t = at8_c.rearrange("q (s v) -> q s v", s=8)
            wt_t = []
            for li, wsrc in enumerate((wt1, wt2)):
                w_ = cpool.tile([128, 128], mybir.dt.float16, name=f"wt{li}")
                nc.sync.dma_start(w_[:, :], wsrc[:, :])
                wt_t.append(w_)
            db1_t = cpool.tile([128, 1024], mybir.dt.float32, name="db1")
            nc.sync.dma_start(db1_t[:, :], db1[:, :])
            db2_t = cpool.tile([128, 1], mybir.dt.float32, name="db2")
            nc.sync.dma_start(db2_t[:, :], db2[:, :])
            dsq_t = cpool.tile([128, N], mybir.dt.float16, name="dsq")
            nc.sync.dma_start(dsq_t[:, :], dsq[:, :])
            dv1_t = cpool.tile([128, N], mybir.dt.float16, name="dv1")
            nc.sync.dma_start(dv1_t[:, :], dv1[:, :])

            stage_A(0, 0)
            # software-pipelined steady state
            for p in range(PAIRS + 1):
                if p + 1 < PAIRS:
                    load_xt(p + 1)
                if p < PAIRS:
                    stage_W(p, 0)
                if p + 1 < PAIRS:
                    stage_A(p + 1, 0)
                if p >= 1:
                    stage_W(p - 1, 1)
                    store_out(p - 1)
                if p < PAIRS:
                    stage_A(p, 1)

    nc.compile()
    return nc


def _host_prep(x, edge_index, W1, b1, W2, b2):
    x = np.ascontiguousarray(np.asarray(x, dtype=np.float32))
    ei = np.asarray(edge_index)
    row, col = ei[0], ei[1]
    deg = np.zeros(N, np.float32)
    np.add.at(deg, col, 1.0)
    deg += 1.0
    dinv = (1.0 / np.sqrt(deg)).astype(np.float32)
    # integer adjacency counts (+ self loops): exact in fp8e4
    Adj = np.zeros((N, N), np.float32)
    np.add.at(Adj, (col, row), 1.0)
    Adj[np.arange(N), np.arange(N)] += 1.0
    at_pad = np.zeros((NPAD, N), np.float32)
    at_pad[:N] = Adj.T
    at8 = np.ascontiguousarray(
        at_pad.reshape(8, 128, N).transpose(1, 0, 2).reshape(128, 8 * N)
    ).astype(F8)

    wts = []
    for W in (W1, W2):
        wt = np.zeros((128, 128), np.float16)
        wt[:64, :64] = np.asarray(W).astype(np.float16)
        wt[64:, 64:] = np.asarray(W).astype(np.float16)
        wts.append(wt)

    # db[p, 128c+f] : bias tiles preloaded into the W-fold psum
    cc = np.arange(8)
    pp = np.arange(128)
    dest = (128 * cc[None, :] + pp[:, None])          # [128, 8]
    dpad = np.zeros(NPAD, np.float32)
    dpad[:N] = dinv
    b1f = np.tile(np.asarray(b1, np.float32), 2)      # [128]
    b2f = np.tile(np.asarray(b2, np.float32), 2)
    db1 = (dpad[dest][:, :, None] * b1f[None, None, :]).reshape(128, 1024)
    db1 = np.ascontiguousarray(db1)
    db2 = np.ascontiguousarray(b2f[:, None])          # [128, 1] per-partition
    dsq = np.ascontiguousarray(
        np.broadcast_to((dinv * dinv)[None, :], (128, N))).astype(np.float16)
    dv1 = np.ascontiguousarray(
        np.broadcast_to(dinv[None, :], (128, N))).astype(np.float16)

    # per-core layer-1 lhsT tiles: [pair, 128, (jj, i, blk, d)] fp8 hi/lo
    slabs = []
    for k in range(NCORES):
        c, hf = k // 2, k % 2
        xs = x[500 * hf:500 * hf + 500, 128 * c:128 * (c + 1), :].reshape(
            64, N, D) * dinv[None, :, None]
        xs_pad = np.zeros((64, NPAD, D), np.float32)
        xs_pad[:, :N] = xs
        # [p, blk, jj, i, pp, d] -> [p, pp, jj, i, blk, d]
        a = xs_pad.reshape(PAIRS, 2, 4, 2, 128, D).transpose(0, 4, 2, 3, 1, 5)
        hi = a.astype(F8)
        if COMP1:
            lo = (a - hi.astype(np.float32)).astype(F8)
        else:
            lo = None
        slabs.append((np.ascontiguousarray(hi.reshape(PAIRS * 128, NPAD)),
                      None if lo is None else
                      np.ascontiguousarray(lo.reshape(PAIRS * 128, NPAD))))
    return at8, wts, db1, db2, dsq, dv1, slabs


def kernel(x, edge_index, W1, b1, W2, b2):
    global _prog, LAST_RESULTS
    if _prog is None:
        _prog = _build_program()
    nc = _prog

    at8, wts, db1, db2, dsq, dv1, slabs = _host_prep(
        x, edge_index, W1, b1, W2, b2)
    in_maps = []
    for k in range(NCORES):
        m = {"xhi": slabs[k][0], "at8": at8,
             "wt1": wts[0], "wt2": wts[1],
             "db1": db1, "db2": db2, "dsq": dsq, "dv1": dv1}
        if COMP1:
            m["xlo"] = slabs[k][1]
        in_maps.append(m)

    LAST_RESULTS = run_bass_kernel_spmd(nc, in_maps, core_ids=list(range(NCORES)))

    out = np.empty((N, T, D), np.float32)
    for k in range(NCORES):
        c, hf = k // 2, k % 2
        # raw transposed [p, (b, d), v] -> [blk=2p+b, v, d]
        raw = np.asarray(LAST_RESULTS.results[k]["xout"]).reshape(PAIRS, 2, D, N)
        r = raw.transpose(0, 1, 3, 2).reshape(64 * N, D)
        r = r.astype(np.float32).reshape(500, CS, D)
        out[500 * hf:500 * hf + 500, 128 * c:128 * (c + 1), :] = r
    return out


# revision 37
# speedup vs baseline: 2.3288x; 1.1849x over previous
"""Trainium2 Bass kernel for ChunkedTGnnModel (2-layer GCN over temporal chunks).

Math: the reference flattens each temporal chunk to a [128000, 64] slab
(row u = node*128 + t_local) while edges are replicated per-timestep with
t-major offsets (tl*N + v). Both live in the same flat index space, so the
per-chunk operator is block-diagonal: 128 consecutive 1000-row blocks of the
slab each get the same dense normalized adjacency A_hat [1000 x 1000]:

    out = relu(blockdiag(A_hat) @ (slab @ W1) + b1)   (then layer 2 same)

Key optimization vs the fp16 version: A_hat = D^-1/2 (Adj + I) D^-1/2 where
Adj+I has small-integer entries that are EXACT in fp8e4 (e4m3). We fold the
left/right D^-1/2 into the operands (host pre-scales X by dinv; dest-side
dinv^2 is folded into the PSUM->SBUF copy before the W-fold), so the big
A-type matmuls run in fp8 DoubleRow mode (256-deep contraction, 0.5
cycles/row = 4x fp16 PE throughput) with NO quantization error from the
adjacency itself. Layer-1 input additionally ships as an fp8 (hi, lo)
mantissa-split pair (costs nothing: host prep), halving its quantization
error; layer-2 re-quantizes on-chip in a single pass (measured end-to-end
rel err ~1.1e-2 vs the 2e-2 gate).

Bias is injected exactly by pre-loading the W-fold PSUM banks with the bias
tile and accumulating the matmuls on top (start=False).

Sharding: 8 cores = 4 chunks x 2 node-halves; each core owns a contiguous
[64000, 64] slab piece (64 blocks = 32 block-pairs). Output is written fp16
and upconverted on host.
"""
import sys
import numpy as np

sys.path.insert(0, '/opt/trn_rl_repo')

import concourse.bass as bass  # noqa: E402
import concourse.bacc as bacc  # noqa: E402
import concourse.mybir as mybir  # noqa: E402
import concourse.tile as tile  # noqa: E402
from concourse.bass_utils import run_bass_kernel_spmd  # noqa: E402

try:
    import ml_dtypes
    F8 = ml_dtypes.float8_e4m3
except ImportError:  # pragma: no cover
    F8 = None

N, T, D = 1000, 512, 64
CS = 128                 # timesteps per chunk
NCORES = 8
ROWS = 64000             # slab rows per core (64 blocks x 1000)
PAIRS = 32
NPAD = 1024              # padded src-node count (8 x 128)
# A-type moving chunks: (psum tile, col offset in tile, global c0, width)
A_CHUNKS = [(0, 0, 0, 256), (0, 256, 256, 256), (1, 0, 512, 256), (1, 256, 768, 232)]

COMP1 = False            # layer-1 fp8 hi+lo compensation (host-side split)
COMP2 = False            # layer-2 on-chip hi+lo compensation
PE_BIAS = True           # layer-1 bias via rank-1 PE matmul vs psum preload

_prog = None
LAST_RESULTS = None

F8D = mybir.dt.float8e4
DR = mybir.MatmulPerfMode.DoubleRow


def _build_program(skip=frozenset()):
    nc = bacc.Bacc(None)
    XW = 2 * NPAD if COMP1 else NPAD
    xin = nc.declare_dram_parameter("xin", [PAIRS * 128, XW], F8D, isOutput=False)
    at8 = nc.declare_dram_parameter("at8", [128, 8 * N], F8D, isOutput=False)
    wt1 = nc.declare_dram_parameter("wt1", [128, 128], mybir.dt.float16, isOutput=False)
    wt2 = nc.declare_dram_parameter("wt2", [128, 128], mybir.dt.float16, isOutput=False)
    db1 = nc.declare_dram_parameter("db1", [128, 1024], mybir.dt.float32, isOutput=False)
    db2 = nc.declare_dram_parameter("db2", [128, 1], mybir.dt.float32, isOutput=False)
    dk1 = nc.declare_dram_parameter("dk1", [4, 256], mybir.dt.float16, isOutput=False)
    bd1 = nc.declare_dram_parameter("bd1", [4, 512], mybir.dt.float16, isOutput=False)
    dsq = nc.declare_dram_parameter("dsq", [128, N], mybir.dt.float16, isOutput=False)
    dv1 = nc.declare_dram_parameter("dv1", [128, N], mybir.dt.float16, isOutput=False)
    # raw per-pair layer-2 output, transposed [pair*128 (blk,d), dest];
    # host un-permutes
    xout = nc.declare_dram_parameter("xout", [PAIRS * 128, N], mybir.dt.float16,
                                     isOutput=True)

    with tile.TileContext(nc) as tc:
        with tc.tile_pool(name="const", bufs=1) as cpool, \
             tc.tile_pool(name="work", bufs=2) as wpool, \
             tc.tile_pool(name="gps_pool", bufs=2, space="PSUM") as gpool, \
             tc.tile_pool(name="hps_pool", bufs=1, space="PSUM") as hpool:

            xin_v = xin.rearrange("(p q) f -> p q f", q=128)
            xout_v = xout.rearrange("(p q) v -> p q v", q=128)

            st = {}

            def load_xt(p):
                xt = wpool.tile([128, XW], F8D, name="xt", tag="xt")
                if "indma" not in skip:
                    nc.sync.dma_start(xt[:, :], xin_v[p])
                if COMP1:
                    st[p] = {"xt": (xt[:, 0:NPAD], xt[:, NPAD:XW])}
                else:
                    st[p] = {"xt": (xt[:, 0:NPAD], None)}

            def stage_A(p, li):
                """fp8 DoubleRow A-type matmuls for layer li -> g psum tiles."""
                if li == 0:
                    xt = st[p]["xt"]
                    srcs = [t_ for t_ in xt if t_ is not None]
                else:
                    srcs = [t_ for t_ in st[p]["h"] if t_ is not None]
                gps = [gpool.tile([128, 512], mybir.dt.float32,
                                  name="gps0", tag="gps0"),
                       gpool.tile([128, 488], mybir.dt.float32,
                                  name="gps1", tag="gps1")]
                nsteps = 4 * len(srcs)
                k = 0
                for src in srcs:
                    sv = src.rearrange("q (jj i f) -> q jj i f", jj=4, i=2)
                    for jj in range(4):
                        for (t_, col, c0, cw) in A_CHUNKS:
                            if "atype" in skip:
                                continue
                            # one accumulation group per PSUM bank: start
                            # zeroes the whole 2KB zero-region, so only the
                            # first matmul of each bank may set it
                            nc.tensor.matmul(
                                gps[t_][:, col:col + cw],
                                sv[:, jj],
                                at8_t[:, 2 * jj:2 * jj + 2, c0:c0 + cw],
                                start=(k == 0 and col == 0),
                                stop=(k == nsteps - 1 and col != 0),
                                perf_mode=DR)
                        k += 1
                st[p][f"g{li}"] = gps

            def stage_W(p, li):
                """scaled psum->sbuf copy, bias, W-folds, relu."""
                gps = st[p].pop(f"g{li}")
                drow = dsq_t if li == 0 else dv1_t
                gsb = wpool.tile([128, N], mybir.dt.float16,
                                 name=f"gsb{li}", tag=f"gsb{li}")
                # fold the dest-side dinv^2 (layer 1) / dinv (layer 2) into
                # the mandatory PSUM->SBUF copy
                if "copies" not in skip:
                    nc.vector.tensor_tensor(gsb[:, 0:512], gps[0][:, :],
                                            drow[:, 0:512], mybir.AluOpType.mult)
                    nc.vector.tensor_tensor(gsb[:, 512:N], gps[1][:, :],
                                            drow[:, 512:N], mybir.AluOpType.mult)

                if li == 0:
                    # row-major W-fold [dest, (blk,f)]; exact bias by psum
                    # preload (dinv[dest]*b1[f]) accumulated by the matmuls
                    hps = [hpool.tile([128, 512], mybir.dt.float32,
                                      name=f"h1_{t}", tag=f"h1_{t}")
                           for t in range(2)]
                    if "preload" not in skip:
                        if PE_BIAS:
                            # bias = dinv[dest]*b1[f] as a K=4 block-diagonal
                            # rank-1 matmul opening each bank's psum group
                            for t in range(2):
                                nc.tensor.matmul(
                                    hps[t][:, :], dk1_t[:, 128 * t:128 * t + 128],
                                    bd1_t[:, :], start=True, stop=False,
                                    skip_group_check=True)
                        else:
                            nc.vector.tensor_copy(hps[0][:, :], db1_t[:, 0:512])
                            nc.scalar.copy(hps[1][:, :], db1_t[:, 512:1024])
                    for c in range(8):
                        rci = 128 if c < 7 else 104
                        t_, o = c // 4, 128 * (c % 4)
                        if "wfold" in skip:
                            continue
                        nc.tensor.matmul(hps[t_][0:rci, o:o + 128],
                                         gsb[:, 128 * c:128 * c + rci],
                                         wt_t[0][:, :],
                                         start=False, stop=(c % 4 == 3),
                                         skip_group_check=True)
                    regions = [(0, 128, 0, 512), (1, 128, 0, 384), (1, 104, 384, 512)]
                    hq = wpool.tile([128, NPAD], F8D, name="hq", tag="hq")
                    # zero the (src >= 1000) pad rows read by layer-2 lhsT
                    nc.gpsimd.memset(hq[96:128, 896:NPAD], 0)
                    for t_, rr, c0r, c1r in regions:
                        if "relu" in skip:
                            continue
                        nc.scalar.activation(hq[0:rr, 512 * t_ + c0r:512 * t_ + c1r],
                                             hps[t_][0:rr, c0r:c1r],
                                             mybir.ActivationFunctionType.Relu)
                    lq = None
                    if COMP2:
                        lq = wpool.tile([128, NPAD], F8D, name="lq", tag="lq")
                        nc.gpsimd.memset(lq[96:128, 896:NPAD], 0)
                        for t_, rr, c0r, c1r in regions:
                            nc.gpsimd.scalar_tensor_tensor(
                                lq[0:rr, 512 * t_ + c0r:512 * t_ + c1r],
                                hps[t_][0:rr, c0r:c1r], 0.0,
                                hq[0:rr, 512 * t_ + c0r:512 * t_ + c1r],
                                op0=mybir.AluOpType.max,
                                op1=mybir.AluOpType.subtract)
                    st[p]["h"] = (hq, lq)
                else:
                    # transposed W-fold: out H.T [(blk,f), dest]; bias b2[f]
                    # is then per-partition and fuses into the ACT relu
                    hps = [hpool.tile([128, 512], mybir.dt.float32,
                                      name="h2_0", tag="h2_0"),
                           hpool.tile([128, 488], mybir.dt.float32,
                                      name="h2_1", tag="h2_1")]
                    if "wfold" not in skip:
                        nc.tensor.matmul(hps[0][:, :], wt_t[1][:, :],
                                         gsb[:, 0:512], start=True, stop=True)
                        nc.tensor.matmul(hps[1][:, :], wt_t[1][:, :],
                                         gsb[:, 512:N], start=True, stop=True)
                    o2 = wpool.tile([128, N], mybir.dt.float16,
                                    name="o2", tag="o2")
                    for t, (c0o, c1o) in enumerate(((0, 512), (512, N))):
                        if "relu" in skip:
                            continue
                        nc.scalar.activation(o2[:, c0o:c1o], hps[t][:, :],
                                             mybir.ActivationFunctionType.Relu,
                                             bias=db2_t[:, 0:1])
                    st[p]["o2"] = o2

            def store_out(p):
                o2 = st[p].pop("o2")
                if "outdma" not in skip:
                    # raw contiguous tile dump; host un-permutes
                    nc.sync.dma_start(xout_v[p], o2[:, :])
                del st[p]

            # prologue: first pair's input first so A-type starts early;
            # constants go over the Pool/SWDGE path to avoid serializing
            # against pair loads on HWDGE
            load_xt(0)
            at8_c = cpool.tile([128, 8 * N], F8D, name="at8")
            nc.sync.dma_start(at8_c[:, 0:4 * N], at8[:, 0:4 * N])
            nc.sync.dma_start(at8_c[:, 4 * N:8 * N], at8[:, 4 * N:8 * N])
            at8_t = at8_c.rearrange("q (s v) -> q s v", s=8)
            wt_t = []
            for li, wsrc in enumerate((wt1, wt2)):
                w_ = cpool.tile([128, 128], mybir.dt.float16, name=f"wt{li}")
                nc.sync.dma_start(w_[:, :], wsrc[:, :])
                wt_t.append(w_)
            db1_t = cpool.tile([128, 1024], mybir.dt.float32, name="db1")
            nc.sync.dma_start(db1_t[:, :], db1[:, :])
            dk1_t = cpool.tile([4, 256], mybir.dt.float16, name="dk1")
            nc.sync.dma_start(dk1_t[:, :], dk1[:, :])
            bd1_t = cpool.tile([4, 512], mybir.dt.float16, name="bd1")
            nc.sync.dma_start(bd1_t[:, :], bd1[:, :])
            db2_t = cpool.tile([128, 1], mybir.dt.float32, name="db2")
            nc.sync.dma_start(db2_t[:, :], db2[:, :])
            dsq_t = cpool.tile([128, N], mybir.dt.float16, name="dsq")
            nc.sync.dma_start(dsq_t[:, :], dsq[:, :])
            dv1_t = cpool.tile([128, N], mybir.dt.float16, name="dv1")
            nc.sync.dma_start(dv1_t[:, :], dv1[:, :])

            stage_A(0, 0)
            # software-pipelined steady state
            for p in range(PAIRS + 1):
                if p + 1 < PAIRS:
                    load_xt(p + 1)
                if p < PAIRS:
                    stage_W(p, 0)
                if p + 1 < PAIRS:
                    stage_A(p + 1, 0)
                if p >= 1:
                    stage_W(p - 1, 1)
                    store_out(p - 1)
                if p < PAIRS:
                    stage_A(p, 1)

    nc.compile()
    return nc


def _host_prep(x, edge_index, W1, b1, W2, b2):
    x = np.ascontiguousarray(np.asarray(x, dtype=np.float32))
    ei = np.asarray(edge_index)
    row, col = ei[0], ei[1]
    deg = np.zeros(N, np.float32)
    np.add.at(deg, col, 1.0)
    deg += 1.0
    dinv = (1.0 / np.sqrt(deg)).astype(np.float32)
    # integer adjacency counts (+ self loops): exact in fp8e4
    Adj = np.zeros((N, N), np.float32)
    np.add.at(Adj, (col, row), 1.0)
    Adj[np.arange(N), np.arange(N)] += 1.0
    at_pad = np.zeros((NPAD, N), np.float32)
    at_pad[:N] = Adj.T
    at8 = np.ascontiguousarray(
        at_pad.reshape(8, 128, N).transpose(1, 0, 2).reshape(128, 8 * N)
    ).astype(F8)

    wts = []
    for W in (W1, W2):
        wt = np.zeros((128, 128), np.float16)
        wt[:64, :64] = np.asarray(W).astype(np.float16)
        wt[64:, 64:] = np.asarray(W).astype(np.float16)
        wts.append(wt)

    # db[p, 128c+f] : bias tiles preloaded into the W-fold psum
    cc = np.arange(8)
    pp = np.arange(128)
    dest = (128 * cc[None, :] + pp[:, None])          # [128, 8]
    dpad = np.zeros(NPAD, np.float32)
    dpad[:N] = dinv
    b1f = np.tile(np.asarray(b1, np.float32), 2)      # [128]
    b2f = np.tile(np.asarray(b2, np.float32), 2)
    db1 = (dpad[dest][:, :, None] * b1f[None, None, :]).reshape(128, 1024)
    db1 = np.ascontiguousarray(db1)
    db2 = np.ascontiguousarray(b2f[:, None])          # [128, 1] per-partition
    # K=4 rank-1 bias operands: dk1[ci, 128t+p] = dinv[128(4t+ci)+p],
    # bd1[ci, 128cj+f] = (ci==cj) * b1f[f]
    dk1 = np.zeros((4, 256), np.float16)
    for t in range(2):
        for ci in range(4):
            dk1[ci, 128 * t:128 * t + 128] = dpad[128 * (4 * t + ci) + pp]
    bd1 = np.zeros((4, 512), np.float16)
    for ci in range(4):
        bd1[ci, 128 * ci:128 * ci + 128] = b1f
    dk1 = np.ascontiguousarray(dk1)
    bd1 = np.ascontiguousarray(bd1)
    dsq = np.ascontiguousarray(
        np.broadcast_to((dinv * dinv)[None, :], (128, N))).astype(np.float16)
    dv1 = np.ascontiguousarray(
        np.broadcast_to(dinv[None, :], (128, N))).astype(np.float16)

    # per-core layer-1 lhsT tiles: [pair, 128, (jj, i, blk, d)] fp8 hi/lo
    slabs = []
    for k in range(NCORES):
        c, hf = k // 2, k % 2
        xs = x[500 * hf:500 * hf + 500, 128 * c:128 * (c + 1), :].reshape(
            64, N, D) * dinv[None, :, None]
        xs_pad = np.zeros((64, NPAD, D), np.float32)
        xs_pad[:, :N] = xs
        # [p, blk, jj, i, pp, d] -> [p, pp, jj, i, blk, d]
        a = xs_pad.reshape(PAIRS, 2, 4, 2, 128, D).transpose(0, 4, 2, 3, 1, 5)
        hi = a.astype(F8).reshape(PAIRS, 128, NPAD)
        if COMP1:
            lo = (a - hi.reshape(PAIRS, 128, 4, 2, 2, D).astype(np.float32)
                  ).astype(F8).reshape(PAIRS, 128, NPAD)
            xq = np.concatenate([hi, lo], axis=2)
        else:
            xq = hi
        slabs.append(np.ascontiguousarray(xq.reshape(PAIRS * 128, -1)))
    return at8, wts, db1, db2, dk1, bd1, dsq, dv1, slabs


def kernel(x, edge_index, W1, b1, W2, b2):
    global _prog, LAST_RESULTS
    if _prog is None:
        _prog = _build_program()
    nc = _prog

    at8, wts, db1, db2, dk1, bd1, dsq, dv1, slabs = _host_prep(
        x, edge_index, W1, b1, W2, b2)
    in_maps = [{"xin": slabs[k], "at8": at8,
                "wt1": wts[0], "wt2": wts[1],
                "db1": db1, "db2": db2, "dk1": dk1, "bd1": bd1,
                "dsq": dsq, "dv1": dv1}
               for k in range(NCORES)]

    LAST_RESULTS = run_bass_kernel_spmd(nc, in_maps, core_ids=list(range(NCORES)))

    out = np.empty((N, T, D), np.float32)
    for k in range(NCORES):
        c, hf = k // 2, k % 2
        # raw transposed [p, (b, d), v] -> [blk=2p+b, v, d]
        raw = np.asarray(LAST_RESULTS.results[k]["xout"]).reshape(PAIRS, 2, D, N)
        r = raw.transpose(0, 1, 3, 2).reshape(64 * N, D)
        r = r.astype(np.float32).reshape(500, CS, D)
        out[500 * hf:500 * hf + 500, 128 * c:128 * (c + 1), :] = r
    return out
